# revision 10
# baseline (speedup 1.0000x reference)
"""Trainium2 Bass kernel for masked sigmoid context attention.

Model (per batch b, with n = R*C = 4096 tokens, D = 512, H = 8 heads of d = 64):
    qh/kh/vh = heads(x @ W + b)
    attn = sigmoid(qh @ kh^T / 8) * mask_keys
    attn = attn / (eps + sum(mask))          # per-batch scalar
    out  = (attn @ vh heads-merged) @ Wo + bo + q

Sharding: 8 cores = 2 batches x 4 head-groups (2 heads / group).
Each core computes its group's projections, flash-style sigmoid attention
(sigmoid is elementwise -> no softmax bookkeeping), and a partial output
projection x_g @ Wo_g.  The host sums the 4 partials per batch and adds
bias + residual (pure unsharding; all matmul FLOPs run on device).

Key device-side choices (see engine docs):
  * masked keys are compacted away on the host (mask is ~50% zeros), and
    mask/denominator are folded into V rows, so masking costs nothing
  * all matmuls in bf16 with fp32 PSUM accumulation (residual dominates the
    output magnitude, so attention-path bf16 error is ~1e-4 relative)
  * q/k/v transposed on-chip via identity-matmul (regular matmul speed)
  * QK^T (K=64) runs 2 heads concurrently via PE row-packing; attn@V (M=64)
    runs 2 heads concurrently via PE col-packing
  * sigmoid on ScalarE from 2-bank PSUM tiles (FD=1024) to amortize overhead
"""

import math
import os
from contextlib import ExitStack

import ml_dtypes
import numpy as np

import concourse.bass as bass
import concourse.mybir as mybir
import concourse.tile as tile
from concourse import bacc
from concourse.bass import ts
from concourse.bass_utils import run_bass_kernel_spmd
from concourse.masks import make_identity

F32 = mybir.dt.float32
BF16 = mybir.dt.bfloat16
BF = ml_dtypes.bfloat16

H = 8
DH = 64
D = 512
GD = 128           # head-group dim = 2 heads x 64
NQ = 4096          # tokens per batch
TEMP = 8.0
EPS = 1e-6
QB = 512           # query block for attention
N_CORES = 8

LAST_RESULT = None  # BassKernelResults of the most recent run (for test harness)
_NC_CACHE = {}


def _build_nc(KT: int) -> bass.Bass:
    """Bass program for one core: batch slice + one head-group. KT = key tiles."""
    KM = KT * 128
    nc = bacc.Bacc(None)

    xq = nc.declare_dram_parameter("xq", [NQ, D], BF16, isOutput=False)
    xk = nc.declare_dram_parameter("xk", [KM, D], BF16, isOutput=False)
    xv = nc.declare_dram_parameter("xv", [KM, D], BF16, isOutput=False)
    wq = nc.declare_dram_parameter("wq", [D, GD], F32, isOutput=False)
    wk = nc.declare_dram_parameter("wk", [D, GD], F32, isOutput=False)
    wv = nc.declare_dram_parameter("wv", [D, GD], F32, isOutput=False)
    wo = nc.declare_dram_parameter("wo", [GD, D], F32, isOutput=False)
    bq = nc.declare_dram_parameter("bq", [GD, 1], F32, isOutput=False)
    bk = nc.declare_dram_parameter("bk", [GD, 1], F32, isOutput=False)
    bv = nc.declare_dram_parameter("bv", [1, GD], F32, isOutput=False)
    # per-key scale = mask/(eps+sum(mask)), in both layouts we need
    vs_p = nc.declare_dram_parameter("vs_p", [KM, 1], F32, isOutput=False)   # key on partition
    vs_f = nc.declare_dram_parameter("vs_f", [1, KM], F32, isOutput=False)   # key on free
    out = nc.declare_dram_parameter("out", [NQ, D], F32, isOutput=True)

    with tile.TileContext(nc) as tc, ExitStack() as ctx:
        const = ctx.enter_context(tc.tile_pool(name="const", bufs=1))
        # persistent per-group tensors
        persist = ctx.enter_context(tc.tile_pool(name="persist", bufs=1))
        raw_pool = ctx.enter_context(tc.tile_pool(name="raw", bufs=6))
        tchunk = ctx.enter_context(tc.tile_pool(name="tchunk", bufs=6))
        p_pool = ctx.enter_context(tc.tile_pool(name="p", bufs=3))
        out_pool = ctx.enter_context(tc.tile_pool(name="outs", bufs=4))
        psum_misc = ctx.enter_context(tc.tile_pool(name="pm", bufs=2, space="PSUM"))
        psum_s = ctx.enter_context(tc.tile_pool(name="ps", bufs=2, space="PSUM"))
        psum_x = ctx.enter_context(tc.tile_pool(name="px", bufs=2, space="PSUM"))

        # ---- constants -------------------------------------------------
        ident = const.tile([128, 128], BF16)
        make_identity(nc, ident[:])

        def load_w_chunks(dram, name):  # (D, GD) -> sbuf (128, 4, GD) bf16
            f = const.tile([128, 4, GD], F32, tag=name + "_st")
            nc.gpsimd.dma_start(f[:], dram.rearrange("(c p) m -> p c m", p=128))
            b = const.tile([128, 4, GD], BF16, tag=name)
            nc.vector.tensor_copy(b[:], f[:])
            return b

        wq_b = load_w_chunks(wq, "wq_b")
        wk_b = load_w_chunks(wk, "wk_b")
        wv_b = load_w_chunks(wv, "wv_b")
        wo_f = const.tile([GD, D], F32, tag="wo_st")
        nc.sync.dma_start(wo_f[:], wo[:, :])
        wo_b = const.tile([GD, D], BF16)
        nc.vector.tensor_copy(wo_b[:], wo_f[:])

        bq_s = const.tile([GD, 1], F32)
        nc.sync.dma_start(bq_s[:], bq[:, :])
        bk_s = const.tile([GD, 1], F32)
        nc.sync.dma_start(bk_s[:], bk[:, :])
        bv_f = const.tile([1, GD], F32)
        nc.sync.dma_start(bv_f[:], bv[:, :])
        bv_b = const.tile([1, GD], BF16)
        nc.vector.tensor_copy(bv_b[:], bv_f[:])

        vsp_s = const.tile([128, KT], F32)
        nc.gpsimd.dma_start(vsp_s[:], vs_p.rearrange("(t p) o -> p (t o)", p=128))
        vsf_f = const.tile([1, KM], F32)
        nc.sync.dma_start(vsf_f[:], vs_f[:, :])
        vsf_b = const.tile([1, KM], BF16)
        nc.vector.tensor_copy(vsf_b[:], vsf_f[:])

        qhT = persist.tile([128, NQ], BF16)   # [h1 d | h2 d] on partitions
        khT = persist.tile([128, KM], BF16)
        vhB = persist.tile([128, KM], BF16)   # per ktile block: (key, group-col)
        xT = persist.tile([128, NQ], BF16)    # attention out, d on partitions

        # ---- transpose helper: 4 ntiles x 4 chunks through one psum bank
        def transpose_group(raws, gs, tag):
            """raws: list of gs sbuf tiles (128, D) bf16 (token-major).
            Returns 4 sbuf tiles (128, gs*128) bf16: chunk c = rows c*128..c*128+127
            of the transposed (D, gs*128) block."""
            outs = []
            for c in range(4):
                pt = psum_misc.tile([128, 512], F32, tag="pm")
                for j in range(gs):
                    nc.tensor.matmul(
                        pt[:, ts(j, 128)], lhsT=raws[j][:, ts(c, 128)],
                        rhs=ident[:], start=True, stop=True)
                st = tchunk.tile([128, 512], BF16, tag=tag)
                nc.vector.tensor_copy(st[:, : gs * 128], pt[:, : gs * 128])
                outs.append(st)
            return outs

        # ---- K path: khT[:, t] = (Wk_g^T @ k^T + bk_g) ------------------
        for g0 in range(0, KT, 4):
            gs = min(4, KT - g0)
            raws = []
            for j in range(gs):
                r = raw_pool.tile([128, D], BF16, tag="kvraw")
                nc.sync.dma_start(r[:], xk[ts(g0 + j, 128), :])
                raws.append(r)
            kTc = transpose_group(raws, gs, "kT")
            pp = psum_misc.tile([128, 512], F32, tag="pm")
            for c in range(4):
                nc.tensor.matmul(
                    pp[:, : gs * 128], lhsT=wk_b[:, c, :], rhs=kTc[c][:, : gs * 128],
                    start=(c == 0), stop=(c == 3))
            nc.vector.tensor_scalar_add(
                khT[:, g0 * 128:(g0 + gs) * 128], pp[:, : gs * 128], bk_s[:])

        # ---- V path: vhB block t = (v_scaled @ Wv_g) + vscale*bv --------
        for g0 in range(0, KT, 4):
            gs = min(4, KT - g0)
            raws = []
            for j in range(gs):
                r0 = raw_pool.tile([128, D], BF16, tag="kvraw")
                nc.sync.dma_start(r0[:], xv[ts(g0 + j, 128), :])
                r = raw_pool.tile([128, D], BF16, tag="vscaled")
                nc.vector.tensor_scalar_mul(r[:], r0[:], vsp_s[:, g0 + j:g0 + j + 1])
                raws.append(r)
            vTc = transpose_group(raws, gs, "vT")
            pv = psum_misc.tile([128, 512], F32, tag="pm")
            for j in range(gs):
                for c in range(4):
                    nc.tensor.matmul(
                        pv[:, ts(j, 128)], lhsT=vTc[c][:, ts(j, 128)],
                        rhs=wv_b[:, c, :], start=(c == 0), stop=False)
                # += vscale[key] * bv  (rank-1 via K=1 matmul)
                nc.tensor.matmul(
                    pv[:, ts(j, 128)], lhsT=vsf_b[:, ts(g0 + j, 128)],
                    rhs=bv_b[:], start=False, stop=True)
            nc.vector.tensor_copy(
                vhB[:, g0 * 128:(g0 + gs) * 128], pv[:, : gs * 128])

        # ---- per query block: project q, attention, output projection --
        for qb in range(NQ // QB):
            qsl = slice(qb * QB, (qb + 1) * QB)
            raws = []
            for j in range(4):
                r = raw_pool.tile([128, D], BF16, tag="qraw")
                nc.sync.dma_start(r[:], xq[ts(qb * 4 + j, 128), :])
                raws.append(r)
            qTc = transpose_group(raws, 4, "qT")
            pp = psum_misc.tile([128, 512], F32, tag="pm")
            for c in range(4):
                nc.tensor.matmul(pp[:], lhsT=wq_b[:, c, :], rhs=qTc[c][:],
                                 start=(c == 0), stop=(c == 3))
            nc.vector.tensor_scalar_add(qhT[:, qsl], pp[:], bq_s[:])

            # attention over all key tiles, accumulating x^T in PSUM
            xa = psum_x.tile([128, QB], F32, tag="px")   # head1 -> partitions 0:64
            xb = psum_x.tile([128, QB], F32, tag="px")   # head2 -> partitions 64:128
            for t in range(KT):
                sg = psum_s.tile([128, 1024], F32, tag="ps")
                # two K=64 matmuls in distinct PE row-groups run concurrently
                nc.tensor.matmul(sg[:, 0:512], lhsT=khT[0:64, ts(t, 128)],
                                 rhs=qhT[0:64, qsl], start=True, stop=True)
                nc.tensor.matmul(sg[:, 512:1024], lhsT=khT[64:128, ts(t, 128)],
                                 rhs=qhT[64:128, qsl], start=True, stop=True)
                p = p_pool.tile([128, 1024], BF16, tag="p")
                nc.scalar.activation(p[:], sg[:], mybir.ActivationFunctionType.Sigmoid,
                                     scale=1.0 / TEMP)
                # two M=64 matmuls in distinct PE col-groups run concurrently
                nc.tensor.matmul(xa[0:64, :], lhsT=vhB[:, t * 128:t * 128 + 64],
                                 rhs=p[:, 0:512], start=(t == 0), stop=(t == KT - 1))
                nc.tensor.matmul(xb[64:128, :], lhsT=vhB[:, t * 128 + 64:t * 128 + 128],
                                 rhs=p[:, 512:1024], start=(t == 0), stop=(t == KT - 1))
            nc.vector.tensor_copy(xT[0:64, qsl], xa[0:64, :])
            nc.vector.tensor_copy(xT[64:128, qsl], xb[64:128, :])

            # partial output projection for this block's 4 token tiles
            for j in range(4):
                nt = qb * 4 + j
                po = psum_misc.tile([128, 512], F32, tag="pm")
                nc.tensor.matmul(po[:], lhsT=xT[:, ts(nt, 128)], rhs=wo_b[:],
                                 start=True, stop=True)
                ot = out_pool.tile([128, D], F32, tag="ot")
                nc.vector.tensor_copy(ot[:], po[:])
                nc.sync.dma_start(out[ts(nt, 128), :], ot[:])

    nc.compile()
    return nc


def kernel(q, k, v, mask, Wq, bq, Wk, bk, Wv, bv, Wo, bo):
    global LAST_RESULT
    q = np.asarray(q, np.float32)
    k = np.asarray(k, np.float32)
    v = np.asarray(v, np.float32)
    mask = np.asarray(mask)
    B, R, C, D_ = q.shape
    n = R * C
    assert (n, D_) == (NQ, D)
    qf = q.reshape(B, n, D)
    kf = k.reshape(B, n, D)
    vf = v.reshape(B, n, D)
    mf = mask.reshape(B, n)
    counts = mf.sum(axis=1)
    KT = max(1, math.ceil(counts.max() / 128))
    KM = KT * 128

    if KT not in _NC_CACHE:
        _NC_CACHE[KT] = _build_nc(KT)
    nc = _NC_CACHE[KT]

    in_maps = []
    kc_b, vc_b, vsp_b, vsf_b, xq_b = [], [], [], [], []
    for b in range(B):
        idx = np.nonzero(mf[b])[0]
        nk = len(idx)
        kc = np.zeros((KM, D), np.float32)
        vc = np.zeros((KM, D), np.float32)
        kc[:nk] = kf[b, idx]
        vc[:nk] = vf[b, idx]
        vs = np.zeros((KM, 1), np.float32)
        vs[:nk] = 1.0 / (EPS + float(counts[b]))
        kc_b.append(kc.astype(BF))
        vc_b.append(vc.astype(BF))
        vsp_b.append(vs)
        vsf_b.append(vs.reshape(1, KM).copy())
        xq_b.append(qf[b].astype(BF))

    Wq = np.asarray(Wq, np.float32)
    Wk = np.asarray(Wk, np.float32)
    Wv = np.asarray(Wv, np.float32)
    Wo = np.asarray(Wo, np.float32)
    bqv = np.asarray(bq, np.float32)
    bkv = np.asarray(bk, np.float32)
    bvv = np.asarray(bv, np.float32)

    for core in range(N_CORES):
        b, g = divmod(core, N_CORES // B)
        gsl = slice(g * GD, (g + 1) * GD)
        in_maps.append(dict(
            xq=xq_b[b], xk=kc_b[b], xv=vc_b[b],
            wq=np.ascontiguousarray(Wq[:, gsl]),
            wk=np.ascontiguousarray(Wk[:, gsl]),
            wv=np.ascontiguousarray(Wv[:, gsl]),
            wo=np.ascontiguousarray(Wo[gsl, :]),
            bq=np.ascontiguousarray(bqv[gsl].reshape(GD, 1)),
            bk=np.ascontiguousarray(bkv[gsl].reshape(GD, 1)),
            bv=np.ascontiguousarray(bvv[gsl].reshape(1, GD)),
            vs_p=vsp_b[b], vs_f=vsf_b[b],
        ))

    LAST_RESULT = run_bass_kernel_spmd(nc, in_maps, list(range(N_CORES)))
    results = LAST_RESULT.results

    bo = np.asarray(bo, np.float32)
    full = np.empty((B, n, D), np.float32)
    for b in range(B):
        acc = results[b * 4 + 0]["out"].astype(np.float32).copy()
        for g in range(1, 4):
            acc += results[b * 4 + g]["out"]
        full[b] = acc + bo[None, :] + qf[b]
    return full.reshape(B, R, C, D).astype(np.float32)


# revision 16
# speedup vs baseline: 1.2108x; 1.2108x over previous
"""Trainium2 Bass kernel for masked sigmoid context attention.

Model (per batch b, with n = R*C = 4096 tokens, D = 512, H = 8 heads of d = 64):
    qh/kh/vh = heads(x @ W + b)
    attn = sigmoid(qh @ kh^T / 8) * mask_keys
    attn = attn / (eps + sum(mask))          # per-batch scalar
    out  = (attn @ vh heads-merged) @ Wo + bo + q

Sharding: 8 cores = 2 batches x 4 head-groups (2 heads / group).
Each core computes its group's projections, flash-style sigmoid attention
(sigmoid is elementwise -> no softmax bookkeeping), and a partial output
projection x_g @ Wo_g.  The host sums the 4 partials per batch and adds
bias + residual (pure unsharding; all matmul FLOPs run on device).

Key device-side choices (see engine docs):
  * masked keys are compacted away on the host (mask is ~50% zeros), and
    mask/denominator are folded into V rows, so masking costs nothing
  * all matmuls in bf16 with fp32 PSUM accumulation (residual dominates the
    output magnitude, so attention-path bf16 error is ~1e-4 relative)
  * q/k/v transposed on-chip via identity-matmul (regular matmul speed)
  * QK^T (K=64) runs 2 heads concurrently via PE row-packing; attn@V (M=64)
    runs 2 heads concurrently via PE col-packing
  * sigmoid on ScalarE from 2-bank PSUM tiles (FD=1024) to amortize overhead
"""

import math
import os
from contextlib import ExitStack

import ml_dtypes
import numpy as np

import concourse.bass as bass
import concourse.mybir as mybir
import concourse.tile as tile
from concourse import bacc
from concourse.bass import ts
from concourse.bass_utils import run_bass_kernel_spmd
from concourse.masks import make_identity

F32 = mybir.dt.float32
BF16 = mybir.dt.bfloat16
BF = ml_dtypes.bfloat16

H = 8
DH = 64
D = 512
GD = 128           # head-group dim = 2 heads x 64
NQ = 4096          # tokens per batch
TEMP = 8.0
EPS = 1e-6
QB = 512           # query block for attention
N_CORES = 8

LAST_RESULT = None  # BassKernelResults of the most recent run (for test harness)
_NC_CACHE = {}


def _build_nc(KT: int) -> bass.Bass:
    """Bass program for one core: batch slice + one head-group. KT = key tiles."""
    KM = KT * 128
    nc = bacc.Bacc(None)

    xq = nc.declare_dram_parameter("xq", [NQ, D], BF16, isOutput=False)
    xk = nc.declare_dram_parameter("xk", [KM, D], BF16, isOutput=False)
    xv = nc.declare_dram_parameter("xv", [KM, D], BF16, isOutput=False)
    wq = nc.declare_dram_parameter("wq", [D, GD], F32, isOutput=False)
    wk = nc.declare_dram_parameter("wk", [D, GD], F32, isOutput=False)
    wv = nc.declare_dram_parameter("wv", [D, GD], F32, isOutput=False)
    wo = nc.declare_dram_parameter("wo", [GD, D], F32, isOutput=False)
    bq = nc.declare_dram_parameter("bq", [GD, 1], F32, isOutput=False)
    bk = nc.declare_dram_parameter("bk", [GD, 1], F32, isOutput=False)
    bv = nc.declare_dram_parameter("bv", [1, GD], F32, isOutput=False)
    # per-key scale = mask/(eps+sum(mask)), in both layouts we need
    vs_p = nc.declare_dram_parameter("vs_p", [KM, 1], F32, isOutput=False)   # key on partition
    vs_f = nc.declare_dram_parameter("vs_f", [1, KM], F32, isOutput=False)   # key on free
    out = nc.declare_dram_parameter("out", [NQ, D], F32, isOutput=True)

    with tile.TileContext(nc) as tc, ExitStack() as ctx:
        const = ctx.enter_context(tc.tile_pool(name="const", bufs=1))
        # persistent per-group tensors
        persist = ctx.enter_context(tc.tile_pool(name="persist", bufs=1))
        raw_pool = ctx.enter_context(tc.tile_pool(name="raw", bufs=10))
        tchunk = ctx.enter_context(tc.tile_pool(name="tchunk", bufs=6))
        p_pool = ctx.enter_context(tc.tile_pool(name="p", bufs=6))
        out_pool = ctx.enter_context(tc.tile_pool(name="outs", bufs=4))

        # ---- constants -------------------------------------------------
        ident = const.tile([128, 128], BF16)
        make_identity(nc, ident[:])

        def load_w_chunks(dram, name):  # (D, GD) -> sbuf (128, 4, GD) bf16
            f = const.tile([128, 4, GD], F32, tag=name + "_st")
            nc.gpsimd.dma_start(f[:], dram.rearrange("(c p) m -> p c m", p=128))
            b = const.tile([128, 4, GD], BF16, tag=name)
            nc.vector.tensor_copy(b[:], f[:])
            return b

        wq_b = load_w_chunks(wq, "wq_b")
        wk_b = load_w_chunks(wk, "wk_b")
        wv_b = load_w_chunks(wv, "wv_b")
        wo_f = const.tile([GD, D], F32, tag="wo_st")
        nc.sync.dma_start(wo_f[:], wo[:, :])
        wo_b = const.tile([GD, D], BF16)
        nc.vector.tensor_copy(wo_b[:], wo_f[:])

        bq_s = const.tile([GD, 1], F32)
        nc.sync.dma_start(bq_s[:], bq[:, :])
        bk_s = const.tile([GD, 1], F32)
        nc.sync.dma_start(bk_s[:], bk[:, :])
        bv_f = const.tile([1, GD], F32)
        nc.sync.dma_start(bv_f[:], bv[:, :])
        bv_b = const.tile([1, GD], BF16)
        nc.vector.tensor_copy(bv_b[:], bv_f[:])

        vsp_s = const.tile([128, KT], F32)
        nc.gpsimd.dma_start(vsp_s[:], vs_p.rearrange("(t p) o -> p (t o)", p=128))
        vsf_f = const.tile([1, KM], F32)
        nc.sync.dma_start(vsf_f[:], vs_f[:, :])
        vsf_b = const.tile([1, KM], BF16)
        nc.vector.tensor_copy(vsf_b[:], vsf_f[:])

        qhT = persist.tile([128, NQ], BF16)   # [h1 d | h2 d] on partitions
        khT = persist.tile([128, KM], BF16)
        vhB = persist.tile([128, KM], BF16)   # per ktile block: (key, group-col)
        xT = persist.tile([128, NQ], BF16)    # attention out, d on partitions

        # ---- helpers ---------------------------------------------------
        def transpose_merged(raws, gs, tag):
            """Transpose gs token-tiles (128, D) via identity-matmul.
            Two D-chunks share one 2-bank PSUM tile -> one wide DVE evac.
            Returns {chunk c: sbuf AP (128, gs*128) bf16} (D-chunk c on
            partitions, tokens on free)."""
            chunks = {}
            for cpair in (0, 2):
                pt = psum_s.tile([128, 1024], F32, tag="ps")
                for ci in range(2):
                    for j in range(gs):
                        nc.tensor.matmul(
                            pt[:, ci * 512 + j * 128: ci * 512 + (j + 1) * 128],
                            lhsT=raws[j][:, ts(cpair + ci, 128)],
                            rhs=ident[:], start=True, stop=True)
                st = tchunk.tile([128, 1024], BF16, tag=tag)
                if gs == 4:
                    nc.vector.tensor_copy(st[:], pt[:])
                else:
                    nc.vector.tensor_copy(
                        st[:].rearrange("p (c w) -> p c w", c=2)[:, :, : gs * 128],
                        pt[:].rearrange("p (c w) -> p c w", c=2)[:, :, : gs * 128])
                for ci in range(2):
                    chunks[cpair + ci] = st[:, ts(ci, 512)]
            return chunks

        def q_prep(qb):
            """DMA + transpose + project queries for block qb into qhT."""
            qsl = slice(qb * QB, (qb + 1) * QB)
            raws = []
            for j in range(4):
                r = raw_pool.tile([128, D], BF16, tag="qraw")
                nc.sync.dma_start(r[:], xq[ts(qb * 4 + j, 128), :])
                raws.append(r)
            qTc = transpose_merged(raws, 4, "qT")
            pp = psum_misc.tile([128, 512], F32, tag="pm")
            for c in range(4):
                nc.tensor.matmul(pp[:], lhsT=wq_b[:, c, :], rhs=qTc[c][:, 0:512],
                                 start=(c == 0), stop=(c == 3))
            nc.vector.tensor_scalar_add(qhT[:, qsl], pp[:], bq_s[:])

        def k_group(g0, gs):
            """khT[:, g0*128:(g0+gs)*128] = Wk_g^T @ k^T + bk_g."""
            raws = []
            for j in range(gs):
                r = raw_pool.tile([128, D], BF16, tag="kvraw")
                nc.sync.dma_start(r[:], xk[ts(g0 + j, 128), :])
                raws.append(r)
            kTc = transpose_merged(raws, gs, "kT")
            pp = psum_misc.tile([128, 512], F32, tag="pm")
            for c in range(4):
                nc.tensor.matmul(
                    pp[:, : gs * 128], lhsT=wk_b[:, c, :],
                    rhs=kTc[c][:, : gs * 128], start=(c == 0), stop=(c == 3))
            nc.vector.tensor_scalar_add(
                khT[:, g0 * 128:(g0 + gs) * 128], pp[:, : gs * 128], bk_s[:])

        def v_group(g0, gs):
            """vhB block t = (v*mask/denom) @ Wv_g + vscale*bv, t in group.
            V loads ride the SWDGE queue to keep the SP load queue short."""
            raws = []
            for j in range(gs):
                r0 = raw_pool.tile([128, D], BF16, tag="kvraw")
                nc.gpsimd.dma_start(r0[:], xv[ts(g0 + j, 128), :])
                r = raw_pool.tile([128, D], BF16, tag="vscaled")
                nc.vector.tensor_scalar_mul(
                    r[:], r0[:], vsp_s[:, g0 + j:g0 + j + 1])
                raws.append(r)
            vTc = transpose_merged(raws, gs, "vT")
            pv = psum_misc.tile([128, 512], F32, tag="pm")
            for j in range(gs):
                for c in range(4):
                    nc.tensor.matmul(
                        pv[:, ts(j, 128)], lhsT=vTc[c][:, ts(j, 128)],
                        rhs=wv_b[:, c, :], start=(c == 0), stop=False)
                # += vscale[key] * bv  (rank-1 via K=1 matmul)
                nc.tensor.matmul(
                    pv[:, ts(j, 128)], lhsT=vsf_b[:, ts(g0 + j, 128)],
                    rhs=bv_b[:], start=False, stop=True)
            nc.vector.tensor_copy(
                vhB[:, g0 * 128:(g0 + gs) * 128], pv[:, : gs * 128])

        def attn_tile(qsl, t, xa, xb):
            sg = psum_s.tile([128, 1024], F32, tag="ps")
            # two K=64 matmuls in distinct PE row-groups run concurrently
            nc.tensor.matmul(sg[:, 0:512], lhsT=khT[0:64, ts(t, 128)],
                             rhs=qhT[0:64, qsl], start=True, stop=True)
            nc.tensor.matmul(sg[:, 512:1024], lhsT=khT[64:128, ts(t, 128)],
                             rhs=qhT[64:128, qsl], start=True, stop=True)
            p = p_pool.tile([128, 1024], BF16, tag="p")
            nc.scalar.activation(p[:], sg[:], mybir.ActivationFunctionType.Sigmoid,
                                 scale=1.0 / TEMP)
            # two M=64 matmuls in distinct PE col-groups run concurrently
            nc.tensor.matmul(xa[0:64, :], lhsT=vhB[:, t * 128:t * 128 + 64],
                             rhs=p[:, 0:512], start=(t == 0), stop=(t == KT - 1))
            nc.tensor.matmul(xb[64:128, :], lhsT=vhB[:, t * 128 + 64:t * 128 + 128],
                             rhs=p[:, 512:1024], start=(t == 0), stop=(t == KT - 1))

        # ---- main: qb0 interleaves K/V prep groups with its attention --
        with tc.tile_pool(name="ps", bufs=2, space="PSUM") as psum_s, \
             tc.tile_pool(name="px", bufs=2, space="PSUM") as psum_x, \
             tc.tile_pool(name="pm", bufs=2, space="PSUM") as psum_misc:
            groups = [(g0, min(4, KT - g0)) for g0 in range(0, KT, 4)]
            q_prep(0)
            for qb in range(NQ // QB):
                qsl = slice(qb * QB, (qb + 1) * QB)
                xa = psum_x.tile([128, QB], F32, tag="px")  # h1 -> parts 0:64
                xb = psum_x.tile([128, QB], F32, tag="px")  # h2 -> parts 64:128
                if qb == 0:
                    # K/V prep streams in; attention consumes each 4-ktile
                    # group as soon as it lands (ACT warms up under prep)
                    for gi, (g0, gs) in enumerate(groups):
                        k_group(g0, gs)
                        v_group(g0, gs)
                        if gi == 0:
                            q_prep(1)
                        for t in range(g0, g0 + gs):
                            attn_tile(qsl, t, xa, xb)
                else:
                    for t in range(KT):
                        attn_tile(qsl, t, xa, xb)
                nc.vector.tensor_copy(xT[0:64, qsl], xa[0:64, :])
                nc.vector.tensor_copy(xT[64:128, qsl], xb[64:128, :])

                # partial output projection; stores ride the SWDGE queue so
                # they never head-of-line-block the SP load queue
                for j in range(4):
                    nt = qb * 4 + j
                    po = psum_misc.tile([128, 512], F32, tag="pm")
                    nc.tensor.matmul(po[:], lhsT=xT[:, ts(nt, 128)], rhs=wo_b[:],
                                     start=True, stop=True)
                    ot = out_pool.tile([128, D], F32, tag="ot")
                    nc.vector.tensor_copy(ot[:], po[:])
                    nc.gpsimd.dma_start(out[ts(nt, 128), :], ot[:])
                if qb + 2 < NQ // QB:
                    q_prep(qb + 2)

    nc.compile()
    return nc


def kernel(q, k, v, mask, Wq, bq, Wk, bk, Wv, bv, Wo, bo):
    global LAST_RESULT
    q = np.asarray(q, np.float32)
    k = np.asarray(k, np.float32)
    v = np.asarray(v, np.float32)
    mask = np.asarray(mask)
    B, R, C, D_ = q.shape
    n = R * C
    assert (n, D_) == (NQ, D)
    qf = q.reshape(B, n, D)
    kf = k.reshape(B, n, D)
    vf = v.reshape(B, n, D)
    mf = mask.reshape(B, n)
    counts = mf.sum(axis=1)
    KT = max(1, math.ceil(counts.max() / 128))
    KM = KT * 128

    if KT not in _NC_CACHE:
        _NC_CACHE[KT] = _build_nc(KT)
    nc = _NC_CACHE[KT]

    in_maps = []
    kc_b, vc_b, vsp_b, vsf_b, xq_b = [], [], [], [], []
    for b in range(B):
        idx = np.nonzero(mf[b])[0]
        nk = len(idx)
        kc = np.zeros((KM, D), np.float32)
        vc = np.zeros((KM, D), np.float32)
        kc[:nk] = kf[b, idx]
        vc[:nk] = vf[b, idx]
        vs = np.zeros((KM, 1), np.float32)
        vs[:nk] = 1.0 / (EPS + float(counts[b]))
        kc_b.append(kc.astype(BF))
        vc_b.append(vc.astype(BF))
        vsp_b.append(vs)
        vsf_b.append(vs.reshape(1, KM).copy())
        xq_b.append(qf[b].astype(BF))

    Wq = np.asarray(Wq, np.float32)
    Wk = np.asarray(Wk, np.float32)
    Wv = np.asarray(Wv, np.float32)
    Wo = np.asarray(Wo, np.float32)
    bqv = np.asarray(bq, np.float32)
    bkv = np.asarray(bk, np.float32)
    bvv = np.asarray(bv, np.float32)

    for core in range(N_CORES):
        b, g = divmod(core, N_CORES // B)
        gsl = slice(g * GD, (g + 1) * GD)
        in_maps.append(dict(
            xq=xq_b[b], xk=kc_b[b], xv=vc_b[b],
            wq=np.ascontiguousarray(Wq[:, gsl]),
            wk=np.ascontiguousarray(Wk[:, gsl]),
            wv=np.ascontiguousarray(Wv[:, gsl]),
            wo=np.ascontiguousarray(Wo[gsl, :]),
            bq=np.ascontiguousarray(bqv[gsl].reshape(GD, 1)),
            bk=np.ascontiguousarray(bkv[gsl].reshape(GD, 1)),
            bv=np.ascontiguousarray(bvv[gsl].reshape(1, GD)),
            vs_p=vsp_b[b], vs_f=vsf_b[b],
        ))

    LAST_RESULT = run_bass_kernel_spmd(nc, in_maps, list(range(N_CORES)))
    results = LAST_RESULT.results

    bo = np.asarray(bo, np.float32)
    full = np.empty((B, n, D), np.float32)
    for b in range(B):
        acc = results[b * 4 + 0]["out"].astype(np.float32).copy()
        for g in range(1, 4):
            acc += results[b * 4 + g]["out"]
        full[b] = acc + bo[None, :] + qf[b]
    return full.reshape(B, R, C, D).astype(np.float32)


# revision 33
# speedup vs baseline: 1.3047x; 1.0776x over previous
"""Trainium2 Bass kernel for masked sigmoid context attention.

Model (per batch b, with n = R*C = 4096 tokens, D = 512, H = 8 heads of d = 64):
    qh/kh/vh = heads(x @ W + b)
    attn = sigmoid(qh @ kh^T / 8) * mask_keys
    attn = attn / (eps + sum(mask))          # per-batch scalar
    out  = (attn @ vh heads-merged) @ Wo + bo + q

Sharding: 8 cores = 2 batches x 4 head-groups (2 heads / group).
Each core computes its group's projections, flash-style sigmoid attention
(sigmoid is elementwise -> no softmax bookkeeping), and a partial output
projection x_g @ Wo_g.  The host sums the 4 partials per batch and adds
bias + residual (pure unsharding; all matmul FLOPs run on device).

Key device-side choices (see engine docs):
  * masked keys are compacted away on the host (mask is ~50% zeros), and
    mask/denominator are folded into V rows, so masking costs nothing
  * all matmuls in bf16 with fp32 PSUM accumulation (residual dominates the
    output magnitude, so attention-path bf16 error is ~1e-4 relative)
  * q/k/v transposed on-chip via identity-matmul (regular matmul speed)
  * QK^T (K=64) runs 2 heads concurrently via PE row-packing; attn@V (M=64)
    runs 2 heads concurrently via PE col-packing
  * sigmoid on ScalarE from 2-bank PSUM tiles (FD=1024) to amortize overhead
"""

import math
import os
from contextlib import ExitStack

import ml_dtypes
import numpy as np

import concourse.bass as bass
import concourse.mybir as mybir
import concourse.tile as tile
from concourse import bacc
from concourse.bass import ts
from concourse.bass_utils import run_bass_kernel_spmd
from concourse.masks import make_identity

F32 = mybir.dt.float32
BF16 = mybir.dt.bfloat16
BF = ml_dtypes.bfloat16

H = 8
DH = 64
D = 512
GD = 128           # head-group dim = 2 heads x 64
NQ = 4096          # tokens per batch
TEMP = 8.0
EPS = 1e-6
QB = 512           # query block for attention
N_CORES = 8

LAST_RESULT = None  # BassKernelResults of the most recent run (for test harness)
_NC_CACHE = {}


def _build_nc(KT: int) -> bass.Bass:
    """Bass program for one core: batch slice + one head-group. KT = key tiles."""
    KM = KT * 128
    nc = bacc.Bacc(None)

    xq = nc.declare_dram_parameter("xq", [NQ, D], BF16, isOutput=False)
    xk = nc.declare_dram_parameter("xk", [KM, D], BF16, isOutput=False)
    xv = nc.declare_dram_parameter("xv", [KM, D], BF16, isOutput=False)
    wq = nc.declare_dram_parameter("wq", [D, GD], F32, isOutput=False)
    wk = nc.declare_dram_parameter("wk", [D, GD], F32, isOutput=False)
    wv = nc.declare_dram_parameter("wv", [D, GD], F32, isOutput=False)
    wo = nc.declare_dram_parameter("wo", [GD, D], F32, isOutput=False)
    bq = nc.declare_dram_parameter("bq", [GD, 1], F32, isOutput=False)
    bk = nc.declare_dram_parameter("bk", [GD, 1], F32, isOutput=False)
    bv = nc.declare_dram_parameter("bv", [1, GD], F32, isOutput=False)
    # per-key scale = mask/(eps+sum(mask)), in both layouts we need
    vs_p = nc.declare_dram_parameter("vs_p", [KM, 1], F32, isOutput=False)   # key on partition
    vs_f = nc.declare_dram_parameter("vs_f", [1, KM], F32, isOutput=False)   # key on free
    out = nc.declare_dram_parameter("out", [NQ, D], F32, isOutput=True)

    with tile.TileContext(nc) as tc, ExitStack() as ctx:
        const = ctx.enter_context(tc.tile_pool(name="const", bufs=1))
        # persistent per-group tensors
        persist = ctx.enter_context(tc.tile_pool(name="persist", bufs=1))
        raw_pool = ctx.enter_context(tc.tile_pool(name="raw", bufs=10))
        tchunk = ctx.enter_context(tc.tile_pool(name="tchunk", bufs=6))
        p_pool = ctx.enter_context(tc.tile_pool(name="p", bufs=6))
        out_pool = ctx.enter_context(tc.tile_pool(name="outs", bufs=4))

        # ---- constants -------------------------------------------------
        ident = const.tile([128, 128], BF16)
        make_identity(nc, ident[:])

        # first key-group loads go ahead of the const loads on the SP queue
        k0_raws = []
        for j in range(min(4, KT)):
            r = raw_pool.tile([128, D], BF16, tag="kvraw")
            nc.sync.dma_start(r[:], xk[ts(j, 128), :])
            k0_raws.append(r)

        def load_w_chunks(dram, name):  # (D, GD) -> sbuf (128, 4, GD) bf16
            f = const.tile([128, 4, GD], F32, tag=name + "_st")
            nc.sync.dma_start(f[:], dram.rearrange("(c p) m -> p c m", p=128))
            b = const.tile([128, 4, GD], BF16, tag=name)
            nc.vector.tensor_copy(b[:], f[:])
            return b

        wq_b = load_w_chunks(wq, "wq_b")
        wk_b = load_w_chunks(wk, "wk_b")
        wv_b = load_w_chunks(wv, "wv_b")
        wo_f = const.tile([GD, D], F32, tag="wo_st")
        nc.sync.dma_start(wo_f[:], wo[:, :])
        wo_b = const.tile([GD, D], BF16)
        nc.vector.tensor_copy(wo_b[:], wo_f[:])

        bq_s = const.tile([GD, 1], F32)
        nc.sync.dma_start(bq_s[:], bq[:, :])
        bk_s = const.tile([GD, 1], F32)
        nc.sync.dma_start(bk_s[:], bk[:, :])
        bv_f = const.tile([1, GD], F32)
        nc.sync.dma_start(bv_f[:], bv[:, :])
        bv_b = const.tile([1, GD], BF16)
        nc.vector.tensor_copy(bv_b[:], bv_f[:])

        vsp_s = const.tile([128, KT], F32)
        nc.sync.dma_start(vsp_s[:], vs_p.rearrange("(t p) o -> p (t o)", p=128))
        vsf_f = const.tile([1, KM], F32)
        nc.sync.dma_start(vsf_f[:], vs_f[:, :])
        vsf_b = const.tile([1, KM], BF16)
        nc.vector.tensor_copy(vsf_b[:], vsf_f[:])

        qhT = persist.tile([128, NQ], BF16)   # [h1 d | h2 d] on partitions
        khT = persist.tile([128, KM], BF16)
        vhB = persist.tile([128, KM], BF16)   # per ktile block: (key, group-col)
        xT = persist.tile([128, NQ], BF16)    # attention out, d on partitions

        # ---- helpers ---------------------------------------------------
        def transpose_merged(pool, raws, gs, tag, evac):
            """Transpose gs token-tiles (128, D) via identity-matmul; two
            D-chunks share one 2-bank PSUM tile -> one wide evac.
            Returns {chunk c: sbuf AP (128, gs*128) bf16}."""
            chunks = {}
            for cpair in (0, 2):
                pt = pool.tile([128, 1024], F32, tag=pool.name + "_t")
                for ci in range(2):
                    for j in range(gs):
                        nc.tensor.matmul(
                            pt[:, ci * 512 + j * 128: ci * 512 + (j + 1) * 128],
                            lhsT=raws[j][:, ts(cpair + ci, 128)],
                            rhs=ident[:], start=True, stop=True)
                st = tchunk.tile([128, 1024], BF16, tag=tag)
                if gs == 4:
                    evac(st[:], pt[:])
                else:
                    evac(st[:].rearrange("p (c w) -> p c w", c=2)[:, :, : gs * 128],
                         pt[:].rearrange("p (c w) -> p c w", c=2)[:, :, : gs * 128])
                for ci in range(2):
                    chunks[cpair + ci] = st[:, ts(ci, 512)]
            return chunks

        def transpose_single(pool, raws, gs, tag, evac):
            """One D-chunk per 1-bank PSUM tile (for the low-duty pm pool)."""
            chunks = {}
            for c in range(4):
                pt = pool.tile([128, 512], F32, tag=pool.name + "_p")
                for j in range(gs):
                    nc.tensor.matmul(
                        pt[:, ts(j, 128)], lhsT=raws[j][:, ts(c, 128)],
                        rhs=ident[:], start=True, stop=True)
                st = tchunk.tile([128, 512], BF16, tag=tag)
                evac(st[:, : gs * 128], pt[:, : gs * 128])
                chunks[c] = st[:]
            return chunks

        dve_copy = nc.vector.tensor_copy
        act_copy = nc.scalar.copy

        def q_prep(tpool, ppool, qb, merged):
            """DMA + transpose + project queries for block qb into qhT."""
            qsl = slice(qb * QB, (qb + 1) * QB)
            raws = []
            for j in range(4):
                r = raw_pool.tile([128, D], BF16, tag="qraw")
                nc.sync.dma_start(r[:], xq[ts(qb * 4 + j, 128), :])
                raws.append(r)
            if merged:
                qTc = transpose_merged(tpool, raws, 4, "qT", dve_copy)
            else:
                qTc = transpose_single(ppool, raws, 4, "qTs", dve_copy)
            pp = ppool.tile([128, 512], F32, tag=ppool.name + "_p")
            for c in range(4):
                nc.tensor.matmul(pp[:], lhsT=wq_b[:, c, :], rhs=qTc[c][:, 0:512],
                                 start=(c == 0), stop=(c == 3))
            nc.vector.tensor_scalar_add(qhT[:, qsl], pp[:], bq_s[:])

        def k_group(tpool, ppool, g0, gs, raws=None):
            """khT[:, g0*128:(g0+gs)*128] = Wk_g^T @ k^T + bk_g.
            Transpose evacs ride ScalarE: in the prep phase attention cannot
            start yet (pool-phase boundary), so ScalarE cycles are free."""
            if raws is None:
                raws = []
                for j in range(gs):
                    r = raw_pool.tile([128, D], BF16, tag="kvraw")
                    nc.sync.dma_start(r[:], xk[ts(g0 + j, 128), :])
                    raws.append(r)
            kTc = transpose_merged(tpool, raws, gs, "kT", act_copy)
            pp = ppool.tile([128, 512], F32, tag=ppool.name + "_p")
            for c in range(4):
                nc.tensor.matmul(
                    pp[:, : gs * 128], lhsT=wk_b[:, c, :],
                    rhs=kTc[c][:, : gs * 128], start=(c == 0), stop=(c == 3))
            nc.vector.tensor_scalar_add(
                khT[:, g0 * 128:(g0 + gs) * 128], pp[:, : gs * 128], bk_s[:])

        def v_group(tpool, ppool, g0, gs):
            """vhB block t = (v*mask/denom) @ Wv_g + vscale*bv, t in group.
            Loads alternate SP/SWDGE queues; evacs ride ScalarE."""
            raws = []
            for j in range(gs):
                r0 = raw_pool.tile([128, D], BF16, tag="kvraw")
                eng = nc.sync if (g0 + j) % 2 == 0 else nc.gpsimd
                eng.dma_start(r0[:], xv[ts(g0 + j, 128), :])
                r = raw_pool.tile([128, D], BF16, tag="vscaled")
                nc.vector.tensor_scalar_mul(
                    r[:], r0[:], vsp_s[:, g0 + j:g0 + j + 1])
                raws.append(r)
            vTc = transpose_merged(tpool, raws, gs, "vT", act_copy)
            pv = ppool.tile([128, 512], F32, tag=ppool.name + "_p")
            for j in range(gs):
                for c in range(4):
                    nc.tensor.matmul(
                        pv[:, ts(j, 128)], lhsT=vTc[c][:, ts(j, 128)],
                        rhs=wv_b[:, c, :], start=(c == 0), stop=False)
                # += vscale[key] * bv  (rank-1 via K=1 matmul)
                nc.tensor.matmul(
                    pv[:, ts(j, 128)], lhsT=vsf_b[:, ts(g0 + j, 128)],
                    rhs=bv_b[:], start=False, stop=True)
            act_copy(vhB[:, g0 * 128:(g0 + gs) * 128], pv[:, : gs * 128])

        # ---- phase 1: K/V prep + first two query blocks (wide psum) ----
        with tc.tile_pool(name="tp1", bufs=3, space="PSUM") as tp1, \
             tc.tile_pool(name="pj1", bufs=2, space="PSUM") as pj1:
            groups = [(g0, min(4, KT - g0)) for g0 in range(0, KT, 4)]
            for gi, (g0, gs) in enumerate(groups):
                k_group(tp1, pj1, g0, gs, raws=k0_raws if gi == 0 else None)
                v_group(tp1, pj1, g0, gs)
                if gi == 0:
                    # q blocks 0/1 early: they gate the first sigmoids of
                    # phase 2; later K/V groups gate nothing until then
                    q_prep(tp1, pj1, 0, merged=True)
                    q_prep(tp1, pj1, 1, merged=True)

        # ---- phase 2: attention + pipelined q-prep + output projection -
        with tc.tile_pool(name="ps", bufs=2, space="PSUM") as psum_s, \
             tc.tile_pool(name="px", bufs=2, space="PSUM") as psum_x, \
             tc.tile_pool(name="pm", bufs=2, space="PSUM") as psum_misc:

            def out_proj(qb):
                # partial output projection; stores ride the SWDGE queue so
                # they never head-of-line-block the SP load queue
                for j in range(4):
                    nt = qb * 4 + j
                    po = psum_misc.tile([128, 512], F32, tag="pm_p")
                    nc.tensor.matmul(po[:], lhsT=xT[:, ts(nt, 128)], rhs=wo_b[:],
                                     start=True, stop=True)
                    ot = out_pool.tile([128, D], F32, tag="ot")
                    nc.vector.tensor_copy(ot[:], po[:])
                    nc.gpsimd.dma_start(out[ts(nt, 128), :], ot[:])

            # outproj(qb) and q_prep(qb+2) are emitted a few tiles INTO
            # block qb+1 so they don't outrank qb+1's first s-matmul fills
            # in the scheduler's priority order (= emission order)
            deferred = None
            for qb in range(NQ // QB):
                qsl = slice(qb * QB, (qb + 1) * QB)
                xa = psum_x.tile([128, QB], F32, tag="px_x")  # h1 -> parts 0:64
                xb = psum_x.tile([128, QB], F32, tag="px_x")  # h2 -> parts 64:128
                for t in range(KT):
                    sg = psum_s.tile([128, 1024], F32, tag="ps_t")
                    # two K=64 matmuls in distinct PE row-groups, concurrent
                    nc.tensor.matmul(sg[:, 0:512], lhsT=khT[0:64, ts(t, 128)],
                                     rhs=qhT[0:64, qsl], start=True, stop=True)
                    nc.tensor.matmul(sg[:, 512:1024], lhsT=khT[64:128, ts(t, 128)],
                                     rhs=qhT[64:128, qsl], start=True, stop=True)
                    p = p_pool.tile([128, 1024], BF16, tag="p")
                    nc.scalar.activation(
                        p[:], sg[:], mybir.ActivationFunctionType.Sigmoid,
                        scale=1.0 / TEMP)
                    # two M=64 matmuls in distinct PE col-groups, concurrent
                    nc.tensor.matmul(
                        xa[0:64, :], lhsT=vhB[:, t * 128:t * 128 + 64],
                        rhs=p[:, 0:512], start=(t == 0), stop=(t == KT - 1))
                    nc.tensor.matmul(
                        xb[64:128, :], lhsT=vhB[:, t * 128 + 64:t * 128 + 128],
                        rhs=p[:, 512:1024], start=(t == 0), stop=(t == KT - 1))
                    if t == 3 and deferred is not None:
                        deferred()
                        deferred = None
                nc.vector.tensor_copy(xT[0:64, qsl], xa[0:64, :])
                nc.vector.tensor_copy(xT[64:128, qsl], xb[64:128, :])

                def make_deferred(qb=qb):
                    def fn():
                        out_proj(qb)
                        if qb + 2 < NQ // QB:
                            q_prep(None, psum_misc, qb + 2, merged=False)
                    return fn
                deferred = make_deferred()
            deferred()

    nc.compile()
    return nc


def kernel(q, k, v, mask, Wq, bq, Wk, bk, Wv, bv, Wo, bo):
    global LAST_RESULT
    q = np.asarray(q, np.float32)
    k = np.asarray(k, np.float32)
    v = np.asarray(v, np.float32)
    mask = np.asarray(mask)
    B, R, C, D_ = q.shape
    n = R * C
    assert (n, D_) == (NQ, D)
    qf = q.reshape(B, n, D)
    kf = k.reshape(B, n, D)
    vf = v.reshape(B, n, D)
    mf = mask.reshape(B, n)
    counts = mf.sum(axis=1)
    KT = max(1, math.ceil(counts.max() / 128))
    KM = KT * 128

    if KT not in _NC_CACHE:
        _NC_CACHE[KT] = _build_nc(KT)
    nc = _NC_CACHE[KT]

    in_maps = []
    kc_b, vc_b, vsp_b, vsf_b, xq_b = [], [], [], [], []
    for b in range(B):
        idx = np.nonzero(mf[b])[0]
        nk = len(idx)
        kc = np.zeros((KM, D), np.float32)
        vc = np.zeros((KM, D), np.float32)
        kc[:nk] = kf[b, idx]
        vc[:nk] = vf[b, idx]
        vs = np.zeros((KM, 1), np.float32)
        vs[:nk] = 1.0 / (EPS + float(counts[b]))
        kc_b.append(kc.astype(BF))
        vc_b.append(vc.astype(BF))
        vsp_b.append(vs)
        vsf_b.append(vs.reshape(1, KM).copy())
        xq_b.append(qf[b].astype(BF))

    Wq = np.asarray(Wq, np.float32)
    Wk = np.asarray(Wk, np.float32)
    Wv = np.asarray(Wv, np.float32)
    Wo = np.asarray(Wo, np.float32)
    bqv = np.asarray(bq, np.float32)
    bkv = np.asarray(bk, np.float32)
    bvv = np.asarray(bv, np.float32)

    for core in range(N_CORES):
        b, g = divmod(core, N_CORES // B)
        gsl = slice(g * GD, (g + 1) * GD)
        in_maps.append(dict(
            xq=xq_b[b], xk=kc_b[b], xv=vc_b[b],
            wq=np.ascontiguousarray(Wq[:, gsl]),
            wk=np.ascontiguousarray(Wk[:, gsl]),
            wv=np.ascontiguousarray(Wv[:, gsl]),
            wo=np.ascontiguousarray(Wo[gsl, :]),
            bq=np.ascontiguousarray(bqv[gsl].reshape(GD, 1)),
            bk=np.ascontiguousarray(bkv[gsl].reshape(GD, 1)),
            bv=np.ascontiguousarray(bvv[gsl].reshape(1, GD)),
            vs_p=vsp_b[b], vs_f=vsf_b[b],
        ))

    global _last_in_maps
    _last_in_maps = in_maps
    LAST_RESULT = run_bass_kernel_spmd(nc, in_maps, list(range(N_CORES)))
    results = LAST_RESULT.results

    bo = np.asarray(bo, np.float32)
    full = np.empty((B, n, D), np.float32)
    for b in range(B):
        acc = results[b * 4 + 0]["out"].astype(np.float32).copy()
        for g in range(1, 4):
            acc += results[b * 4 + g]["out"]
        full[b] = acc + bo[None, :] + qf[b]
    return full.reshape(B, R, C, D).astype(np.float32)


# revision 39
# speedup vs baseline: 1.3306x; 1.0198x over previous
"""Trainium2 Bass kernel for masked sigmoid context attention.

Model (per batch b, with n = R*C = 4096 tokens, D = 512, H = 8 heads of d = 64):
    qh/kh/vh = heads(x @ W + b)
    attn = sigmoid(qh @ kh^T / 8) * mask_keys
    attn = attn / (eps + sum(mask))          # per-batch scalar
    out  = (attn @ vh heads-merged) @ Wo + bo + q

Sharding: 8 cores = 2 batches x 4 head-groups (2 heads / group).
Each core computes its group's projections, flash-style sigmoid attention
(sigmoid is elementwise -> no softmax bookkeeping), and a partial output
projection x_g @ Wo_g.  The host sums the 4 partials per batch and adds
bias + residual (pure unsharding; all matmul FLOPs run on device).

Key device-side choices (see engine docs):
  * masked keys are compacted away on the host (mask is ~50% zeros), and
    mask/denominator are folded into V rows, so masking costs nothing
  * all matmuls in bf16 with fp32 PSUM accumulation (residual dominates the
    output magnitude, so attention-path bf16 error is ~1e-4 relative)
  * q/k/v transposed on-chip via identity-matmul (regular matmul speed)
  * QK^T (K=64) runs 2 heads concurrently via PE row-packing; attn@V (M=64)
    runs 2 heads concurrently via PE col-packing
  * sigmoid on ScalarE from 2-bank PSUM tiles (FD=1024) to amortize overhead
"""

import math
import os
from contextlib import ExitStack

import ml_dtypes
import numpy as np

import concourse.bass as bass
import concourse.mybir as mybir
import concourse.tile as tile
from concourse import bacc
from concourse.bass import ts
from concourse.bass_utils import run_bass_kernel_spmd
from concourse.masks import make_identity

F32 = mybir.dt.float32
BF16 = mybir.dt.bfloat16
BF = ml_dtypes.bfloat16

H = 8
DH = 64
D = 512
GD = 128           # head-group dim = 2 heads x 64
NQ = 4096          # tokens per batch
TEMP = 8.0
EPS = 1e-6
QB = 512           # query block for attention
N_CORES = 8

LAST_RESULT = None  # BassKernelResults of the most recent run (for test harness)
_NC_CACHE = {}


def _build_nc(KT: int, loop_n: int | None = None) -> bass.Bass:
    """Bass program for one core: batch slice + one head-group. KT = key tiles.

    loop_n: benchmarking aid -- wrap the whole kernel body in a hardware
    For_i loop so one NEFF execution runs the kernel loop_n times (used to
    measure per-iteration HW time through the remote-dispatch jitter).
    Not used for the normal kernel() path."""
    KM = KT * 128
    nc = bacc.Bacc(None)

    xq = nc.declare_dram_parameter("xq", [NQ, D], BF16, isOutput=False)
    xk = nc.declare_dram_parameter("xk", [KM, D], BF16, isOutput=False)
    xv = nc.declare_dram_parameter("xv", [KM, D], BF16, isOutput=False)
    wq = nc.declare_dram_parameter("wq", [D, GD], F32, isOutput=False)
    wk = nc.declare_dram_parameter("wk", [D, GD], F32, isOutput=False)
    wv = nc.declare_dram_parameter("wv", [D, GD], F32, isOutput=False)
    wo = nc.declare_dram_parameter("wo", [GD, D], F32, isOutput=False)
    bq = nc.declare_dram_parameter("bq", [GD, 1], F32, isOutput=False)
    bk = nc.declare_dram_parameter("bk", [GD, 1], F32, isOutput=False)
    bv = nc.declare_dram_parameter("bv", [1, GD], F32, isOutput=False)
    # per-key scale = mask/(eps+sum(mask)), in both layouts we need
    vs_p = nc.declare_dram_parameter("vs_p", [KM, 1], F32, isOutput=False)   # key on partition
    vs_f = nc.declare_dram_parameter("vs_f", [1, KM], F32, isOutput=False)   # key on free
    out = nc.declare_dram_parameter("out", [NQ, D], F32, isOutput=True)

    with tile.TileContext(nc) as tc, ExitStack() as ctx:
        if loop_n is not None:
            ctx.enter_context(tc.For_i(0, loop_n, 1))
        const = ctx.enter_context(tc.tile_pool(name="const", bufs=1))
        # persistent per-group tensors
        persist = ctx.enter_context(tc.tile_pool(name="persist", bufs=1))
        raw_pool = ctx.enter_context(tc.tile_pool(name="raw", bufs=16))
        tchunk = ctx.enter_context(tc.tile_pool(name="tchunk", bufs=6))
        p_pool = ctx.enter_context(tc.tile_pool(name="p", bufs=6))
        out_pool = ctx.enter_context(tc.tile_pool(name="outs", bufs=4))

        # ---- constants -------------------------------------------------
        ident = const.tile([128, 128], BF16)
        make_identity(nc, ident[:])

        # first key-group loads go ahead of the const loads on the SP queue
        # (they gate the first transposes -> evacs -> sigmoids)
        k0_raws = []
        for j in range(min(4, KT)):
            r = raw_pool.tile([128, D], BF16, tag="kvraw")
            nc.sync.dma_start(r[:], xk[ts(j, 128), :])
            k0_raws.append(r)

        def load_w_chunks(dram, name):  # (D, GD) -> sbuf (128, 4, GD) bf16
            f = const.tile([128, 4, GD], F32, tag=name + "_st")
            nc.sync.dma_start(f[:], dram.rearrange("(c p) m -> p c m", p=128))
            b = const.tile([128, 4, GD], BF16, tag=name)
            nc.vector.tensor_copy(b[:], f[:])
            return b

        wq_b = load_w_chunks(wq, "wq_b")
        wk_b = load_w_chunks(wk, "wk_b")
        wv_b = load_w_chunks(wv, "wv_b")
        wo_f = const.tile([GD, D], F32, tag="wo_st")
        nc.sync.dma_start(wo_f[:], wo[:, :])
        wo_b = const.tile([GD, D], BF16)
        nc.vector.tensor_copy(wo_b[:], wo_f[:])

        bq_s = const.tile([GD, 1], F32)
        nc.sync.dma_start(bq_s[:], bq[:, :])
        bk_s = const.tile([GD, 1], F32)
        nc.sync.dma_start(bk_s[:], bk[:, :])
        bv_f = const.tile([1, GD], F32)
        nc.sync.dma_start(bv_f[:], bv[:, :])
        bv_b = const.tile([1, GD], BF16)
        nc.vector.tensor_copy(bv_b[:], bv_f[:])

        vsp_s = const.tile([128, KT], F32)
        nc.sync.dma_start(vsp_s[:], vs_p.rearrange("(t p) o -> p (t o)", p=128))
        vsf_f = const.tile([1, KM], F32)
        nc.sync.dma_start(vsf_f[:], vs_f[:, :])
        vsf_b = const.tile([1, KM], BF16)
        nc.vector.tensor_copy(vsf_b[:], vsf_f[:])

        qhT = persist.tile([128, NQ], BF16)   # [h1 d | h2 d] on partitions
        khT = persist.tile([128, KM], BF16)
        vhB = persist.tile([128, KM], BF16)   # per ktile block: (key, group-col)
        xT = persist.tile([128, NQ], BF16)    # attention out, d on partitions

        # ---- helpers ---------------------------------------------------
        def transpose_merged(pool, raws, gs, tag, evac):
            """Transpose gs token-tiles (128, D) via identity-matmul; two
            D-chunks share one 2-bank PSUM tile -> one wide evac.
            Returns {chunk c: sbuf AP (128, gs*128) bf16}."""
            chunks = {}
            for cpair in (0, 2):
                pt = pool.tile([128, 1024], F32, tag=pool.name + "_t")
                for ci in range(2):
                    for j in range(gs):
                        nc.tensor.matmul(
                            pt[:, ci * 512 + j * 128: ci * 512 + (j + 1) * 128],
                            lhsT=raws[j][:, ts(cpair + ci, 128)],
                            rhs=ident[:], start=True, stop=True)
                st = tchunk.tile([128, 1024], BF16, tag=tag)
                if gs == 4:
                    evac(st[:], pt[:])
                else:
                    evac(st[:].rearrange("p (c w) -> p c w", c=2)[:, :, : gs * 128],
                         pt[:].rearrange("p (c w) -> p c w", c=2)[:, :, : gs * 128])
                for ci in range(2):
                    chunks[cpair + ci] = st[:, ts(ci, 512)]
            return chunks

        def transpose_single(pool, raws, gs, tag, evac):
            """One D-chunk per 1-bank PSUM tile (for the low-duty pm pool)."""
            chunks = {}
            for c in range(4):
                pt = pool.tile([128, 512], F32, tag=pool.name + "_p")
                for j in range(gs):
                    nc.tensor.matmul(
                        pt[:, ts(j, 128)], lhsT=raws[j][:, ts(c, 128)],
                        rhs=ident[:], start=True, stop=True)
                st = tchunk.tile([128, 512], BF16, tag=tag)
                evac(st[:, : gs * 128], pt[:, : gs * 128])
                chunks[c] = st[:]
            return chunks

        dve_copy = nc.vector.tensor_copy
        act_copy = nc.scalar.copy

        def q_prep(tpool, ppool, qb, merged):
            """DMA + transpose + project queries for block qb into qhT."""
            qsl = slice(qb * QB, (qb + 1) * QB)
            raws = []
            for j in range(4):
                r = raw_pool.tile([128, D], BF16, tag="qraw")
                nc.sync.dma_start(r[:], xq[ts(qb * 4 + j, 128), :])
                raws.append(r)
            if merged:
                qTc = transpose_merged(tpool, raws, 4, "qT", dve_copy)
            else:
                qTc = transpose_single(ppool, raws, 4, "qTs", dve_copy)
            pp = ppool.tile([128, 512], F32, tag=ppool.name + "_p")
            for c in range(4):
                nc.tensor.matmul(pp[:], lhsT=wq_b[:, c, :], rhs=qTc[c][:, 0:512],
                                 start=(c == 0), stop=(c == 3))
            nc.vector.tensor_scalar_add(qhT[:, qsl], pp[:], bq_s[:])

        def k_group(tpool, ppool, g0, gs, raws=None):
            """khT[:, g0*128:(g0+gs)*128] = Wk_g^T @ k^T + bk_g.
            Transpose evacs ride ScalarE: in the prep phase attention cannot
            start yet (pool-phase boundary), so ScalarE cycles are free."""
            if raws is None:
                raws = []
                for j in range(gs):
                    r = raw_pool.tile([128, D], BF16, tag="kvraw")
                    nc.sync.dma_start(r[:], xk[ts(g0 + j, 128), :])
                    raws.append(r)
            kTc = transpose_merged(tpool, raws, gs, "kT", act_copy)
            pp = ppool.tile([128, 512], F32, tag=ppool.name + "_p")
            for c in range(4):
                nc.tensor.matmul(
                    pp[:, : gs * 128], lhsT=wk_b[:, c, :],
                    rhs=kTc[c][:, : gs * 128], start=(c == 0), stop=(c == 3))
            nc.vector.tensor_scalar_add(
                khT[:, g0 * 128:(g0 + gs) * 128], pp[:, : gs * 128], bk_s[:])

        def v_group(tpool, ppool, g0, gs):
            """vhB block t = (v*mask/denom) @ Wv_g + vscale*bv, t in group.
            Loads alternate SP/SWDGE queues; evacs ride ScalarE."""
            raws = []
            for j in range(gs):
                r0 = raw_pool.tile([128, D], BF16, tag="kvraw")
                eng = nc.sync if (g0 + j) % 2 == 0 else nc.gpsimd
                eng.dma_start(r0[:], xv[ts(g0 + j, 128), :])
                r = raw_pool.tile([128, D], BF16, tag="vscaled")
                nc.vector.tensor_scalar_mul(
                    r[:], r0[:], vsp_s[:, g0 + j:g0 + j + 1])
                raws.append(r)
            vTc = transpose_merged(tpool, raws, gs, "vT", act_copy)
            pv = ppool.tile([128, 512], F32, tag=ppool.name + "_p")
            for j in range(gs):
                for c in range(4):
                    nc.tensor.matmul(
                        pv[:, ts(j, 128)], lhsT=vTc[c][:, ts(j, 128)],
                        rhs=wv_b[:, c, :], start=(c == 0), stop=False)
                # += vscale[key] * bv  (rank-1 via K=1 matmul)
                nc.tensor.matmul(
                    pv[:, ts(j, 128)], lhsT=vsf_b[:, ts(g0 + j, 128)],
                    rhs=bv_b[:], start=False, stop=True)
            act_copy(vhB[:, g0 * 128:(g0 + gs) * 128], pv[:, : gs * 128])

        # ---- phase 1: K/V prep + first two query blocks (wide psum) ----
        with tc.tile_pool(name="tp1", bufs=3, space="PSUM") as tp1, \
             tc.tile_pool(name="pj1", bufs=2, space="PSUM") as pj1:
            groups = [(g0, min(4, KT - g0)) for g0 in range(0, KT, 4)]
            for gi, (g0, gs) in enumerate(groups):
                k_group(tp1, pj1, g0, gs, raws=k0_raws if gi == 0 else None)
                v_group(tp1, pj1, g0, gs)
                if gi == 0:
                    # q blocks 0/1 early: they gate the first sigmoids of
                    # phase 2; later K/V groups gate nothing until then
                    q_prep(tp1, pj1, 0, merged=True)
                    q_prep(tp1, pj1, 1, merged=True)

        # ---- phase 2: attention + pipelined q-prep + output projection -
        with tc.tile_pool(name="ps", bufs=2, space="PSUM") as psum_s, \
             tc.tile_pool(name="px", bufs=2, space="PSUM") as psum_x, \
             tc.tile_pool(name="pm", bufs=2, space="PSUM") as psum_misc:

            def out_proj(qb):
                # partial output projection; stores ride the SWDGE queue so
                # they never head-of-line-block the SP load queue.  The last
                # block's stores are on the critical tail and the SP queue
                # is drained by then, so they go HWDGE instead.
                last = qb == NQ // QB - 1
                for j in range(4):
                    nt = qb * 4 + j
                    po = psum_misc.tile([128, 512], F32, tag="pm_p")
                    nc.tensor.matmul(po[:], lhsT=xT[:, ts(nt, 128)], rhs=wo_b[:],
                                     start=True, stop=True)
                    ot = out_pool.tile([128, D], F32, tag="ot")
                    nc.vector.tensor_copy(ot[:], po[:])
                    (nc.sync if last else nc.gpsimd).dma_start(
                        out[ts(nt, 128), :], ot[:])

            # outproj(qb) and q_prep(qb+2) are emitted a few tiles INTO
            # block qb+1 so they don't outrank qb+1's first s-matmul fills
            # in the scheduler's priority order (= emission order)
            deferred = None
            for qb in range(NQ // QB):
                qsl = slice(qb * QB, (qb + 1) * QB)
                xa = psum_x.tile([128, QB], F32, tag="px_x")  # h1 -> parts 0:64
                xb = psum_x.tile([128, QB], F32, tag="px_x")  # h2 -> parts 64:128
                for t in range(KT):
                    sg = psum_s.tile([128, 1024], F32, tag="ps_t")
                    # two K=64 matmuls in distinct PE row-groups, concurrent
                    nc.tensor.matmul(sg[:, 0:512], lhsT=khT[0:64, ts(t, 128)],
                                     rhs=qhT[0:64, qsl], start=True, stop=True)
                    nc.tensor.matmul(sg[:, 512:1024], lhsT=khT[64:128, ts(t, 128)],
                                     rhs=qhT[64:128, qsl], start=True, stop=True)
                    p = p_pool.tile([128, 1024], BF16, tag="p")
                    nc.scalar.activation(
                        p[:], sg[:], mybir.ActivationFunctionType.Sigmoid,
                        scale=1.0 / TEMP)
                    # two M=64 matmuls in distinct PE col-groups, concurrent
                    nc.tensor.matmul(
                        xa[0:64, :], lhsT=vhB[:, t * 128:t * 128 + 64],
                        rhs=p[:, 0:512], start=(t == 0), stop=(t == KT - 1))
                    nc.tensor.matmul(
                        xb[64:128, :], lhsT=vhB[:, t * 128 + 64:t * 128 + 128],
                        rhs=p[:, 512:1024], start=(t == 0), stop=(t == KT - 1))
                    if t == 3 and deferred is not None:
                        deferred()
                        deferred = None
                nc.vector.tensor_copy(xT[0:64, qsl], xa[0:64, :])
                nc.vector.tensor_copy(xT[64:128, qsl], xb[64:128, :])

                def make_deferred(qb=qb):
                    def fn():
                        out_proj(qb)
                        if qb + 2 < NQ // QB:
                            q_prep(None, psum_misc, qb + 2, merged=False)
                    return fn
                deferred = make_deferred()
            deferred()

    nc.compile()
    return nc


def kernel(q, k, v, mask, Wq, bq, Wk, bk, Wv, bv, Wo, bo):
    global LAST_RESULT
    q = np.asarray(q, np.float32)
    k = np.asarray(k, np.float32)
    v = np.asarray(v, np.float32)
    mask = np.asarray(mask)
    B, R, C, D_ = q.shape
    n = R * C
    assert (n, D_) == (NQ, D)
    qf = q.reshape(B, n, D)
    kf = k.reshape(B, n, D)
    vf = v.reshape(B, n, D)
    mf = mask.reshape(B, n)
    counts = mf.sum(axis=1)
    KT = max(1, math.ceil(counts.max() / 128))
    KM = KT * 128

    if KT not in _NC_CACHE:
        _NC_CACHE[KT] = _build_nc(KT)
    nc = _NC_CACHE[KT]

    in_maps = []
    kc_b, vc_b, vsp_b, vsf_b, xq_b = [], [], [], [], []
    for b in range(B):
        idx = np.nonzero(mf[b])[0]
        nk = len(idx)
        kc = np.zeros((KM, D), np.float32)
        vc = np.zeros((KM, D), np.float32)
        kc[:nk] = kf[b, idx]
        vc[:nk] = vf[b, idx]
        vs = np.zeros((KM, 1), np.float32)
        vs[:nk] = 1.0 / (EPS + float(counts[b]))
        kc_b.append(kc.astype(BF))
        vc_b.append(vc.astype(BF))
        vsp_b.append(vs)
        vsf_b.append(vs.reshape(1, KM).copy())
        xq_b.append(qf[b].astype(BF))

    Wq = np.asarray(Wq, np.float32)
    Wk = np.asarray(Wk, np.float32)
    Wv = np.asarray(Wv, np.float32)
    Wo = np.asarray(Wo, np.float32)
    bqv = np.asarray(bq, np.float32)
    bkv = np.asarray(bk, np.float32)
    bvv = np.asarray(bv, np.float32)

    for core in range(N_CORES):
        b, g = divmod(core, N_CORES // B)
        gsl = slice(g * GD, (g + 1) * GD)
        in_maps.append(dict(
            xq=xq_b[b], xk=kc_b[b], xv=vc_b[b],
            wq=np.ascontiguousarray(Wq[:, gsl]),
            wk=np.ascontiguousarray(Wk[:, gsl]),
            wv=np.ascontiguousarray(Wv[:, gsl]),
            wo=np.ascontiguousarray(Wo[gsl, :]),
            bq=np.ascontiguousarray(bqv[gsl].reshape(GD, 1)),
            bk=np.ascontiguousarray(bkv[gsl].reshape(GD, 1)),
            bv=np.ascontiguousarray(bvv[gsl].reshape(1, GD)),
            vs_p=vsp_b[b], vs_f=vsf_b[b],
        ))

    global _last_in_maps
    _last_in_maps = in_maps
    LAST_RESULT = run_bass_kernel_spmd(nc, in_maps, list(range(N_CORES)))
    results = LAST_RESULT.results

    bo = np.asarray(bo, np.float32)
    full = np.empty((B, n, D), np.float32)
    for b in range(B):
        acc = results[b * 4 + 0]["out"].astype(np.float32).copy()
        for g in range(1, 4):
            acc += results[b * 4 + g]["out"]
        full[b] = acc + bo[None, :] + qf[b]
    return full.reshape(B, R, C, D).astype(np.float32)


# revision 50
# speedup vs baseline: 1.4365x; 1.0796x over previous
"""Trainium2 Bass kernel for masked sigmoid context attention.

Model (per batch b, with n = R*C = 4096 tokens, D = 512, H = 8 heads of d = 64):
    qh/kh/vh = heads(x @ W + b)
    attn = sigmoid(qh @ kh^T / 8) * mask_keys
    attn = attn / (eps + sum(mask))          # per-batch scalar
    out  = (attn @ vh heads-merged) @ Wo + bo + q

Sharding: 8 cores = 2 batches x 4 head-groups (2 heads / group).
Each core computes its group's projections, flash-style sigmoid attention
(sigmoid is elementwise -> no softmax bookkeeping), and a partial output
projection x_g @ Wo_g.  The host sums the 4 partials per batch and adds
bias + residual (pure unsharding; all matmul FLOPs run on device).

Key device-side choices (see engine docs):
  * masked keys are compacted away on the host (mask is ~50% zeros), and
    mask/denominator are folded into V rows, so masking costs nothing
  * all matmuls in bf16 with fp32 PSUM accumulation (residual dominates the
    output magnitude, so attention-path bf16 error is ~1e-4 relative)
  * q/k/v ship host-transposed (contraction dim on rows) so projections
    consume them directly -- zero on-chip transposes
  * QK^T (K=64) runs 2 heads concurrently via PE row-packing; attn@V (M=64)
    runs 2 heads concurrently via PE col-packing (measured concurrent on HW)
  * sigmoid on ScalarE from 2-bank PSUM tiles (FD=1024) to amortize overhead;
    ScalarE is the bottleneck engine (~142us/core) and runs ~97% utilized
"""

import math
import os
from contextlib import ExitStack

import ml_dtypes
import numpy as np

import concourse.bass as bass
import concourse.mybir as mybir
import concourse.tile as tile
from concourse import bacc
from concourse.bass import ts
from concourse.bass_utils import run_bass_kernel_spmd

F32 = mybir.dt.float32
BF16 = mybir.dt.bfloat16
BF = ml_dtypes.bfloat16

H = 8
DH = 64
D = 512
GD = 128           # head-group dim = 2 heads x 64
NQ = 4096          # tokens per batch
TEMP = 8.0
EPS = 1e-6
QB = 512           # query block for attention
N_CORES = 8

LAST_RESULT = None  # BassKernelResults of the most recent run (for test harness)
_NC_CACHE = {}


def _build_nc(KT: int, loop_n: int | None = None) -> bass.Bass:
    """Bass program for one core: batch slice + one head-group. KT = key tiles.

    loop_n: benchmarking aid -- wrap the whole kernel body in a hardware
    For_i loop so one NEFF execution runs the kernel loop_n times (used to
    measure per-iteration HW time through the remote-dispatch jitter).
    Not used for the normal kernel() path."""
    KM = KT * 128
    nc = bacc.Bacc(None)

    # q/k/v arrive HOST-TRANSPOSED (contraction dim D on rows) so the
    # projections can consume them directly -- no on-chip transposes
    xq = nc.declare_dram_parameter("xq", [D, NQ], BF16, isOutput=False)
    xk = nc.declare_dram_parameter("xk", [D, KM], BF16, isOutput=False)
    xv = nc.declare_dram_parameter("xv", [D, KM], BF16, isOutput=False)
    wq = nc.declare_dram_parameter("wq", [D, GD], F32, isOutput=False)
    wk = nc.declare_dram_parameter("wk", [D, GD], F32, isOutput=False)
    wv = nc.declare_dram_parameter("wv", [D, GD], F32, isOutput=False)
    wo = nc.declare_dram_parameter("wo", [GD, D], F32, isOutput=False)
    bq = nc.declare_dram_parameter("bq", [GD, 1], F32, isOutput=False)
    bk = nc.declare_dram_parameter("bk", [GD, 1], F32, isOutput=False)
    bv = nc.declare_dram_parameter("bv", [1, GD], F32, isOutput=False)
    # per-key scale = mask/(eps+sum(mask))
    vs_p = nc.declare_dram_parameter("vs_p", [KM, 1], F32, isOutput=False)   # key on partition
    out = nc.declare_dram_parameter("out", [NQ, D], F32, isOutput=True)

    with tile.TileContext(nc) as tc, ExitStack() as ctx:
        if loop_n is not None:
            ctx.enter_context(tc.For_i(0, loop_n, 1))
        const = ctx.enter_context(tc.tile_pool(name="const", bufs=1))
        persist = ctx.enter_context(tc.tile_pool(name="persist", bufs=1))
        p_pool = ctx.enter_context(tc.tile_pool(name="p", bufs=6))
        out_pool = ctx.enter_context(tc.tile_pool(name="outs", bufs=4))
        psum_s = ctx.enter_context(tc.tile_pool(name="ps", bufs=2, space="PSUM"))
        psum_x = ctx.enter_context(tc.tile_pool(name="px", bufs=2, space="PSUM"))
        psum_misc = ctx.enter_context(tc.tile_pool(name="pm", bufs=2, space="PSUM"))

        # ---- inputs: transposed q/k/v as (128, 4chunk, n) sbuf tiles ----
        # k/v first halves lead the SP queue (they gate the first sigmoids);
        # big tails follow; v rides the SWDGE queue in parallel
        def load_tails():
            # big streaming tails go AFTER the weight/const loads so they
            # don't block the first projections in queue order
            if KM > KH:
                nc.sync.dma_start(xk_s[:, :, KH:KM], xkr[:, :, KH:KM])
                nc.gpsimd.dma_start(xv_s[:, :, KH:KM], xvr[:, :, KH:KM])
            for h in range(1, 4):
                nc.sync.dma_start(xq_s[:, :, ts(h, 2 * QB)], xqr[:, :, ts(h, 2 * QB)])

        # ---- constants --------------------------------------------------
        def load_w_chunks(dram, name):  # (D, GD) -> sbuf (128, 4, GD) bf16
            f = const.tile([128, 4, GD], F32, tag=name + "_st")
            nc.sync.dma_start(f[:], dram.rearrange("(c p) m -> p c m", p=128))
            b = const.tile([128, 4, GD], BF16, tag=name)
            nc.vector.tensor_copy(b[:], f[:])
            return b

        KH = min(4, KT) * 128
        xk_s = persist.tile([128, 4, KM], BF16)
        xv_s = persist.tile([128, 4, KM], BF16)
        xq_s = persist.tile([128, 4, NQ], BF16)
        xkr = xk.rearrange("(c p) n -> p c n", p=128)
        xvr = xv.rearrange("(c p) n -> p c n", p=128)
        xqr = xq.rearrange("(c p) n -> p c n", p=128)
        nc.sync.dma_start(xk_s[:, :, 0:KH], xkr[:, :, 0:KH])
        nc.gpsimd.dma_start(xv_s[:, :, 0:KH], xvr[:, :, 0:KH])
        nc.sync.dma_start(xq_s[:, :, 0:2 * QB], xqr[:, :, 0:2 * QB])

        wq_b = load_w_chunks(wq, "wq_b")
        wk_b = load_w_chunks(wk, "wk_b")
        wv_b = load_w_chunks(wv, "wv_b")
        wo_f = const.tile([GD, D], F32, tag="wo_st")
        nc.sync.dma_start(wo_f[:], wo[:, :])
        wo_b = const.tile([GD, D], BF16)
        nc.vector.tensor_copy(wo_b[:], wo_f[:])

        bq_s = const.tile([GD, 1], F32)
        nc.sync.dma_start(bq_s[:], bq[:, :])
        bk_s = const.tile([GD, 1], F32)
        nc.sync.dma_start(bk_s[:], bk[:, :])
        bv_f = const.tile([1, GD], F32)
        nc.sync.dma_start(bv_f[:], bv[:, :])
        bv_b = const.tile([1, GD], BF16)
        nc.vector.tensor_copy(bv_b[:], bv_f[:])
        ones1 = const.tile([1, 128], BF16)
        nc.gpsimd.memset(ones1[:], 1.0)

        vsp_s = const.tile([128, KT], F32)
        nc.sync.dma_start(vsp_s[:], vs_p.rearrange("(t p) o -> p (t o)", p=128))
        load_tails()

        qhT = persist.tile([128, NQ], BF16)   # [h1 d | h2 d] on partitions
        khT = persist.tile([128, KM], BF16)
        vhB = persist.tile([128, KM], BF16)   # per ktile block: (key, group-col)
        xT = persist.tile([128, NQ], BF16)    # attention out, d on partitions

        # ---- projections (the host-transposed inputs feed PE directly) -
        def q_proj(qb):
            qsl = slice(qb * QB, (qb + 1) * QB)
            pp = psum_misc.tile([128, 512], F32, tag="pm_p")
            for c in range(4):
                nc.tensor.matmul(pp[:], lhsT=wq_b[:, c, :], rhs=xq_s[:, c, qsl],
                                 start=(c == 0), stop=(c == 3))
            nc.vector.tensor_scalar_add(qhT[:, qsl], pp[:], bq_s[:])

        def k_proj(g0, gs):
            ksl = slice(g0 * 128, (g0 + gs) * 128)
            pp = psum_misc.tile([128, 512], F32, tag="pm_p")
            for c in range(4):
                nc.tensor.matmul(pp[:, : gs * 128], lhsT=wk_b[:, c, :],
                                 rhs=xk_s[:, c, ksl], start=(c == 0), stop=(c == 3))
            nc.vector.tensor_scalar_add(khT[:, ksl], pp[:, : gs * 128], bk_s[:])

        def v_proj(g0, gs):
            # vhB[key, :] = ((v @ Wv_g) + bv) * vscale[key]; the row scale
            # commutes with the right-multiplication, so it rides the evac
            pv = psum_misc.tile([128, 512], F32, tag="pm_p")
            for j in range(gs):
                t = g0 + j
                for c in range(4):
                    nc.tensor.matmul(
                        pv[:, ts(j, 128)], lhsT=xv_s[:, c, ts(t, 128)],
                        rhs=wv_b[:, c, :], start=(c == 0), stop=False)
                # += 1 * bv  (rank-1 via K=1 matmul)
                nc.tensor.matmul(pv[:, ts(j, 128)], lhsT=ones1[:],
                                 rhs=bv_b[:], start=False, stop=True)
            for j in range(gs):
                t = g0 + j
                nc.vector.tensor_scalar_mul(
                    vhB[:, ts(t, 128)], pv[:, ts(j, 128)], vsp_s[:, t:t + 1])

        groups = [(g0, min(4, KT - g0)) for g0 in range(0, KT, 4)]
        q_proj(0)
        q_proj(1)
        for g0, gs in groups:
            k_proj(g0, gs)
            v_proj(g0, gs)

        # ---- attention + pipelined q-proj + output projection ----------
        def out_proj(qb):
            # partial output projection; stores ride the SWDGE queue so
            # they never head-of-line-block the SP load queue.  The last
            # block's stores are on the critical tail and the SP queue is
            # drained by then, so they go HWDGE instead.
            last = qb == NQ // QB - 1
            for j in range(4):
                nt = qb * 4 + j
                po = psum_misc.tile([128, 512], F32, tag="pm_p")
                nc.tensor.matmul(po[:], lhsT=xT[:, ts(nt, 128)], rhs=wo_b[:],
                                 start=True, stop=True)
                ot = out_pool.tile([128, D], F32, tag="ot")
                nc.vector.tensor_copy(ot[:], po[:])
                (nc.sync if last else nc.gpsimd).dma_start(
                    out[ts(nt, 128), :], ot[:])

        # outproj(qb) and q_proj(qb+2) are emitted a few tiles INTO block
        # qb+1 so they don't outrank qb+1's first s-matmul fills in the
        # scheduler's priority order (= emission order)
        deferred = None
        for qb in range(NQ // QB):
            qsl = slice(qb * QB, (qb + 1) * QB)
            xa = psum_x.tile([128, QB], F32, tag="px_x")  # h1 -> parts 0:64
            xb = psum_x.tile([128, QB], F32, tag="px_x")  # h2 -> parts 64:128
            for t in range(KT):
                sg = psum_s.tile([128, 1024], F32, tag="ps_t")
                # two K=64 matmuls in distinct PE row-groups, concurrent
                nc.tensor.matmul(sg[:, 0:512], lhsT=khT[0:64, ts(t, 128)],
                                 rhs=qhT[0:64, qsl], start=True, stop=True)
                nc.tensor.matmul(sg[:, 512:1024], lhsT=khT[64:128, ts(t, 128)],
                                 rhs=qhT[64:128, qsl], start=True, stop=True)
                p = p_pool.tile([128, 1024], BF16, tag="p")
                nc.scalar.activation(
                    p[:], sg[:], mybir.ActivationFunctionType.Sigmoid,
                    scale=1.0 / TEMP)
                # two M=64 matmuls in distinct PE col-groups, concurrent
                nc.tensor.matmul(
                    xa[0:64, :], lhsT=vhB[:, t * 128:t * 128 + 64],
                    rhs=p[:, 0:512], start=(t == 0), stop=(t == KT - 1))
                nc.tensor.matmul(
                    xb[64:128, :], lhsT=vhB[:, t * 128 + 64:t * 128 + 128],
                    rhs=p[:, 512:1024], start=(t == 0), stop=(t == KT - 1))
                if t == 3 and deferred is not None:
                    deferred()
                    deferred = None
            nc.vector.tensor_copy(xT[0:64, qsl], xa[0:64, :])
            nc.vector.tensor_copy(xT[64:128, qsl], xb[64:128, :])

            def make_deferred(qb=qb):
                def fn():
                    out_proj(qb)
                    if qb + 2 < NQ // QB:
                        q_proj(qb + 2)
                return fn
            deferred = make_deferred()
        deferred()

    nc.compile()
    return nc


def kernel(q, k, v, mask, Wq, bq, Wk, bk, Wv, bv, Wo, bo):
    global LAST_RESULT
    q = np.asarray(q, np.float32)
    k = np.asarray(k, np.float32)
    v = np.asarray(v, np.float32)
    mask = np.asarray(mask)
    B, R, C, D_ = q.shape
    n = R * C
    assert (n, D_) == (NQ, D)
    qf = q.reshape(B, n, D)
    kf = k.reshape(B, n, D)
    vf = v.reshape(B, n, D)
    mf = mask.reshape(B, n)
    counts = mf.sum(axis=1)
    KT = max(1, math.ceil(counts.max() / 128))
    KM = KT * 128

    if KT not in _NC_CACHE:
        _NC_CACHE[KT] = _build_nc(KT)
    nc = _NC_CACHE[KT]

    in_maps = []
    kc_b, vc_b, vsp_b, xq_b = [], [], [], []
    for b in range(B):
        idx = np.nonzero(mf[b])[0]
        nk = len(idx)
        kc = np.zeros((KM, D), np.float32)
        vc = np.zeros((KM, D), np.float32)
        kc[:nk] = kf[b, idx]
        vc[:nk] = vf[b, idx]
        vs = np.zeros((KM, 1), np.float32)
        vs[:nk] = 1.0 / (EPS + float(counts[b]))
        # ship transposed (contraction dim on rows) so the device consumes
        # them directly as matmul operands -- no on-chip transposes
        kc_b.append(np.ascontiguousarray(kc.astype(BF).T))
        vc_b.append(np.ascontiguousarray(vc.astype(BF).T))
        vsp_b.append(vs)
        xq_b.append(np.ascontiguousarray(qf[b].astype(BF).T))

    Wq = np.asarray(Wq, np.float32)
    Wk = np.asarray(Wk, np.float32)
    Wv = np.asarray(Wv, np.float32)
    Wo = np.asarray(Wo, np.float32)
    bqv = np.asarray(bq, np.float32)
    bkv = np.asarray(bk, np.float32)
    bvv = np.asarray(bv, np.float32)

    for core in range(N_CORES):
        b, g = divmod(core, N_CORES // B)
        gsl = slice(g * GD, (g + 1) * GD)
        in_maps.append(dict(
            xq=xq_b[b], xk=kc_b[b], xv=vc_b[b],
            wq=np.ascontiguousarray(Wq[:, gsl]),
            wk=np.ascontiguousarray(Wk[:, gsl]),
            wv=np.ascontiguousarray(Wv[:, gsl]),
            wo=np.ascontiguousarray(Wo[gsl, :]),
            bq=np.ascontiguousarray(bqv[gsl].reshape(GD, 1)),
            bk=np.ascontiguousarray(bkv[gsl].reshape(GD, 1)),
            bv=np.ascontiguousarray(bvv[gsl].reshape(1, GD)),
            vs_p=vsp_b[b],
        ))

    global _last_in_maps
    _last_in_maps = in_maps
    LAST_RESULT = run_bass_kernel_spmd(nc, in_maps, list(range(N_CORES)))
    results = LAST_RESULT.results

    bo = np.asarray(bo, np.float32)
    full = np.empty((B, n, D), np.float32)
    for b in range(B):
        acc = results[b * 4 + 0]["out"].astype(np.float32).copy()
        for g in range(1, 4):
            acc += results[b * 4 + g]["out"]
        full[b] = acc + bo[None, :] + qf[b]
    return full.reshape(B, R, C, D).astype(np.float32)
